# revision 2
# baseline (speedup 1.0000x reference)
"""Per-env MLP (EnvironVectorField) Trainium2 kernel, v2.

Reference computation (fp32):
    x = u.reshape(B, E, D)  # B=16384, E=8 envs, D=64
    h = swish(x @ W1[e] + b1[e]); h = swish(h @ W2[e] + b2[e])
    h = swish(h @ W3[e] + b3[e]); out = h @ W4[e] + b4[e]
    return out.reshape(B*E, D)

Sharding: expert-parallel — core e computes env e entirely (u rows e::8).

v2 design:
- All device tensors are feature-major. The host ships x already
  transposed to (D, B) and reads y back as (D, B), so the tensor engine
  does zero transposes (v1 spent 32K PE cycles/core on identity-matmul
  transposes).
- Everything computes in bf16 (weights + activations), accumulating in
  fp32 PSUM. Host pre-casts weights, so there are no staging copies.
- L1 has K=64: two m-tiles run CONCURRENTLY in the PE array via row
  tiling (tile_position (0,0)/(64,0)), with x duplicated into SBUF
  partitions 64..127. 2x faster than zero-padding K to 128.
- L4 has M=64: the two 512-column halves of each chunk run concurrently
  via column tiling (tile_position (0,0)/(0,64)), writing partitions
  0..63 / 64..127 of one PSUM bank. 2x faster than half-empty matmuls.
- The batch is processed in chunks of NB columns. Layer 1 of chunk c+1
  is interleaved into layer 3 of chunk c so its PSUM evacuations never
  gate the tensor engine.
"""

import sys

sys.path.insert(0, '/opt/trn_rl_repo')

from contextlib import ExitStack

import ml_dtypes
import numpy as np

import concourse.bacc as bacc
import concourse.bass as bass
import concourse.mybir as mybir
import concourse.tile as tile
from concourse.bass_utils import run_bass_kernel_spmd

F32 = mybir.dt.float32
BF16 = mybir.dt.bfloat16
SILU = mybir.ActivationFunctionType.Silu

N_ENV = 8
D = 64          # state dim
H = 1024        # hidden dim
B = 16384       # rows per env
NB = 1024       # batch-chunk columns per chunk
NCH = B // NB   # chunks
NT = NB // 512  # 512-wide matmul n-tiles per chunk
KT = H // 128   # k/m tiles of 128 over the hidden dim


def build_module(iters: int = 1):
    nc = bacc.Bacc("TRN2", target_bir_lowering=False, num_devices=N_ENV)

    xin = nc.dram_tensor("x", (D, B), BF16, kind="ExternalInput")
    w1 = nc.dram_tensor("w1", (128, H), BF16, kind="ExternalInput")     # rows 0:64 = W1, 64:128 = W1
    w2 = nc.dram_tensor("w2", (128, KT, H), BF16, kind="ExternalInput")  # [ki, ko, M]
    w3 = nc.dram_tensor("w3", (128, KT, H), BF16, kind="ExternalInput")
    w4 = nc.dram_tensor("w4", (128, KT, D), BF16, kind="ExternalInput")
    b1 = nc.dram_tensor("b1", (128, KT), F32, kind="ExternalInput")      # [mi, mo]
    b2 = nc.dram_tensor("b2", (128, KT), F32, kind="ExternalInput")
    b3 = nc.dram_tensor("b3", (128, KT), F32, kind="ExternalInput")
    b4 = nc.dram_tensor("b4", (128, 1), F32, kind="ExternalInput")       # rows 0:64 = b4, 64:128 = b4
    yout = nc.dram_tensor("y", (D, B), F32, kind="ExternalOutput")

    xv = xin.rearrange("d (c n) -> c d n", n=NB)
    yv = yout.rearrange("d (c n) -> c d n", n=NB)

    with tile.TileContext(nc) as tc, ExitStack() as ctx:
        wpool = ctx.enter_context(tc.tile_pool(name="wpool", bufs=1))
        mps = ctx.enter_context(tc.tile_pool(name="mps", bufs=4, space="PSUM"))

        xT0 = wpool.tile([128, NB], BF16)
        xT1 = wpool.tile([128, NB], BF16)

        def dma_in(c, it=0):
            xT = (xT0, xT1)[c % 2]
            nc.sync.dma_start(xT[0:D, :], xv[c])
            nc.sync.dma_start(xT[D:128, :], xv[c])

        if iters == 1:
            # queue chunk-0 input ahead of the 4.5MB weight DMAs (FIFO queues)
            dma_in(0)

        # biases in one padded tile
        ball = wpool.tile([128, 3 * KT + 1], F32)
        nc.sync.dma_start(ball[:, 0:KT], b1[:])
        nc.sync.dma_start(ball[:, KT:2 * KT], b2[:])
        nc.sync.dma_start(ball[:, 2 * KT:3 * KT], b3[:])
        nc.sync.dma_start(ball[:, 3 * KT:3 * KT + 1], b4[:])
        b1s = ball[:, 0:KT]
        b2s = ball[:, KT:2 * KT]
        b3s = ball[:, 2 * KT:3 * KT]
        b4s = ball[:, 3 * KT:3 * KT + 1]

        w1r = wpool.tile([128, H], BF16)
        w2r = wpool.tile([128, KT, H], BF16)
        w3r = wpool.tile([128, KT, H], BF16)
        w4r = wpool.tile([128, KT, D], BF16)
        nc.sync.dma_start(w1r[:], w1[:])
        nc.sync.dma_start(w2r[:], w2[:])
        nc.sync.dma_start(w3r[:], w3[:])
        nc.sync.dma_start(w4r[:], w4[:])

        # persistent activation buffers (fixed roles)
        hA = wpool.tile([128, KT, NB], BF16)   # L1 out
        hB = wpool.tile([128, KT, NB], BF16)   # L2 out
        hC = wpool.tile([128, KT, NB], BF16)   # L3 out
        oT0 = wpool.tile([D, NB], F32)         # L4 out
        oT1 = wpool.tile([D, NB], F32)

        def l1_pair(c, mp, it=0):
            # two m-tiles concurrently: row tiles (0,0) and (64,0), K=64 each
            xT = (xT0, xT1)[c % 2]
            mA, mB = 2 * mp, 2 * mp + 1
            for n in range(NT):
                ns_ = slice(n * 512, (n + 1) * 512)
                pa = mps.tile([128, 512], F32, tag=f"mm{n}",
                              name=f"p1a_{it}_{c}_{mp}_{n}")
                pb = mps.tile([128, 512], F32, tag=f"mm{n}",
                              name=f"p1b_{it}_{c}_{mp}_{n}")
                nc.tensor.matmul(pa[:], w1r[0:D, mA * 128:(mA + 1) * 128],
                                 xT[0:D, ns_], start=True, stop=True,
                                 tile_position=(0, 0))
                nc.tensor.matmul(pb[:], w1r[D:128, mB * 128:(mB + 1) * 128],
                                 xT[D:128, ns_], start=True, stop=True,
                                 tile_position=(64, 0))
                nc.scalar.activation(hA[:, mA, ns_], pa[:],
                                     SILU, bias=b1s[:, mA:mA + 1])
                nc.scalar.activation(hA[:, mB, ns_], pb[:],
                                     SILU, bias=b1s[:, mB:mB + 1])

        def mid_group(li, wr, bs, hs, hd, c, m, it=0):
            pms = [mps.tile([128, 512], F32, tag=f"mm{n}",
                            name=f"p{li}_{it}_{c}_{m}_{n}")[:] for n in range(NT)]
            for k in range(KT):
                for n in range(NT):
                    nc.tensor.matmul(pms[n], wr[:, k, m * 128:(m + 1) * 128],
                                     hs[:, k, n * 512:(n + 1) * 512],
                                     start=(k == 0), stop=(k == KT - 1))
            for n in range(NT):
                nc.scalar.activation(hd[:, m, n * 512:(n + 1) * 512], pms[n],
                                     SILU, bias=bs[:, m:m + 1])

        def tail(c, it=0):
            # L4: hC -> oT, two 512-col halves concurrently via column tiling
            oT = (oT0, oT1)[c % 2]
            p4 = mps.tile([128, 512], F32, tag="mm0", name=f"p4_{it}_{c}")
            for k in range(KT):
                nc.tensor.matmul(p4[0:D, :], w4r[:, k, :], hC[:, k, 0:512],
                                 start=(k == 0), stop=(k == KT - 1),
                                 tile_position=(0, 0))
                nc.tensor.matmul(p4[D:128, :], w4r[:, k, :], hC[:, k, 512:1024],
                                 start=(k == 0), stop=(k == KT - 1),
                                 tile_position=(0, 64))
            nc.vector.tensor_scalar_add(oT[:, 0:512], p4[0:D, :], b4s[0:D])
            nc.vector.tensor_scalar_add(oT[:, 512:1024], p4[D:128, :], b4s[D:128])
            nc.sync.dma_start(yv[c], oT[:])

        def full_pass(it=0):
            if iters != 1:
                dma_in(0, it)
            for mp in range(KT // 2):
                l1_pair(0, mp, it)
            for c in range(NCH):
                if c + 1 < NCH:
                    dma_in(c + 1, it)
                for m in range(KT):
                    mid_group(2, w2r, b2s, hA, hB, c, m, it)
                for m in range(KT):
                    mid_group(3, w3r, b3s, hB, hC, c, m, it)
                    if c + 1 < NCH and m % 2 == 1:
                        l1_pair(c + 1, m // 2, it)
                tail(c, it)

        if iters == 1:
            full_pass()
        else:
            with tc.For_i(0, iters, 1):
                full_pass()

    nc.compile()
    return nc


def _prep_in_maps(t, u, W1, b1, W2, b2, W3, b3, W4, b4):
    bf = ml_dtypes.bfloat16
    u3 = np.asarray(u, np.float32).reshape(B, N_ENV, D)
    in_maps = []
    for e in range(N_ENV):
        w1p = np.empty((128, H), bf)
        w1p[:D] = W1[e].astype(bf)
        w1p[D:] = W1[e].astype(bf)
        b4p = np.empty((128, 1), np.float32)
        b4p[:D, 0] = b4[e]
        b4p[D:, 0] = b4[e]
        in_maps.append({
            "x": u3[:, e, :].T.astype(bf),
            "w1": w1p,
            "w2": W2[e].reshape(KT, 128, H).transpose(1, 0, 2).astype(bf),
            "w3": W3[e].reshape(KT, 128, H).transpose(1, 0, 2).astype(bf),
            "w4": W4[e].reshape(KT, 128, D).transpose(1, 0, 2).astype(bf),
            "b1": np.ascontiguousarray(b1[e].reshape(KT, 128).T.astype(np.float32)),
            "b2": np.ascontiguousarray(b2[e].reshape(KT, 128).T.astype(np.float32)),
            "b3": np.ascontiguousarray(b3[e].reshape(KT, 128).T.astype(np.float32)),
            "b4": b4p,
        })
    return in_maps


_CACHED_NC = None


def kernel(t, u, W1, b1, W2, b2, W3, b3, W4, b4):
    global _CACHED_NC
    u = np.asarray(u, np.float32)
    args = [np.asarray(a, np.float32) for a in (W1, b1, W2, b2, W3, b3, W4, b4)]
    if _CACHED_NC is None:
        _CACHED_NC = build_module()
    in_maps = _prep_in_maps(None, u, *args)
    res = run_bass_kernel_spmd(_CACHED_NC, in_maps, core_ids=list(range(N_ENV)))
    out = np.empty((B * N_ENV, D), np.float32)
    for e in range(N_ENV):
        out[e::N_ENV] = np.asarray(res.results[e]["y"], np.float32).T
    return out
